# revision 9
# baseline (speedup 1.0000x reference)
"""Trainium2 Bass kernel for nn_Pointnet2DetHead (segment_reduce).

Pipeline per core (N sharded 8 ways, 12544 points/core in 98 chunks of 128):
  - box-membership mask[n,p] via exact fp32 compares:
      x-dim on DVE (tensor_scalar is_le/is_ge -> {0,2} int16)
      y/z dims on ACT (Sign(coord - lo), Sign(hi - coord) -> {-1,0,1} int16)
      combined with int16 scalar_tensor_tensor adds + threshold >= 7
  - sums[p,r] and counts[p] via float32r matmuls accumulated in PSUM
  - AllReduce partials across the 8 cores
  - replicated head: mean -> linear heads -> softmax(cls, axis=c) *
    softmax(obj, axis=p)
"""

import sys

if "/opt/trn_rl_repo" not in sys.path:
    sys.path.insert(0, "/opt/trn_rl_repo")

import numpy as np
from contextlib import ExitStack

import concourse.bass as bass
import concourse.tile as tile
import concourse.mybir as mybir
from concourse import bacc
from concourse.bass_utils import run_bass_kernel_spmd
from concourse.mybir import AluOpType as Op, ActivationFunctionType as Act, AxisListType

N_CORES = 8
N, P, R, CO = 100000, 256, 256, 21  # CO = C+1 output classes
CHUNK = 128
N_PAD_CORE = 12544            # 98 chunks of 128
N_CHUNKS = N_PAD_CORE // CHUNK
GROUP = 7                     # chunks per feats DMA group
N_GROUPS = N_CHUNKS // GROUP
FP = mybir.dt.float32
FPR = mybir.dt.float32r
I16 = mybir.dt.int16
BF = mybir.dt.bfloat16

_cache = {}


def _build():
    nc = bacc.Bacc("TRN2", num_devices=N_CORES, target_bir_lowering=False, debug=False)

    feats_d = nc.dram_tensor("feats", [CHUNK, N_CHUNKS * 2 * R], BF, kind="ExternalInput")
    xyzT_d = nc.dram_tensor("xyzT", [14, N_PAD_CORE], FPR, kind="ExternalInput")
    rb_d = nc.dram_tensor("rb", [14, 3 * P], FPR, kind="ExternalInput")
    wcat_d = nc.dram_tensor("wcat", [2 * CHUNK, 64], FPR, kind="ExternalInput")
    bcat_d = nc.dram_tensor("bcat", [64, 1], FP, kind="ExternalInput")
    ident_d = nc.dram_tensor("ident", [CHUNK, CHUNK], FP, kind="ExternalInput")
    ones_d = nc.dram_tensor("ones", [CHUNK, 1], BF, kind="ExternalInput")
    out_d = nc.dram_tensor("out", [P, CO], FP, kind="ExternalOutput")

    with ExitStack() as ctx:
        tc = ctx.enter_context(tile.TileContext(nc))
        const = ctx.enter_context(tc.tile_pool(name="const", bufs=1))
        fpool = ctx.enter_context(tc.tile_pool(name="fpool", bufs=4))
        cmp_p = ctx.enter_context(tc.tile_pool(name="cmp", bufs=6))
        mpool = ctx.enter_context(tc.tile_pool(name="mp", bufs=6))
        spool = ctx.enter_context(tc.tile_pool(name="sp", bufs=2))
        psA = ctx.enter_context(tc.tile_pool(name="psA", bufs=1, space="PSUM"))
        psT = ctx.enter_context(tc.tile_pool(name="psT", bufs=1, space="PSUM"))
        psS = ctx.enter_context(tc.tile_pool(name="psS", bufs=1, space="PSUM"))
        dram = ctx.enter_context(tc.tile_pool(name="dram", bufs=1, space="DRAM"))

        # ---- constants ----
        xyzT = const.tile([14, N_PAD_CORE], FPR)  # [xh,xl,xh, yh,yl,yh, zh,zl,zh, 1,1]
        nc.gpsimd.dma_start(xyzT[:], xyzT_d.ap()[:])
        rb = const.tile([14, 3 * P], FPR)         # scaled-bound matmul rhs
        nc.gpsimd.dma_start(rb[:], rb_d.ap()[:])
        wc0 = const.tile([CHUNK, 64], FPR)
        wc1 = const.tile([CHUNK, 64], FPR)
        nc.gpsimd.dma_start(wc0[:], wcat_d.ap()[0:CHUNK, :])
        nc.gpsimd.dma_start(wc1[:], wcat_d.ap()[CHUNK : 2 * CHUNK, :])
        bcat = const.tile([64, 1], FP)
        nc.gpsimd.dma_start(bcat[:], bcat_d.ap()[:])
        ident = const.tile([CHUNK, CHUNK], FP)
        nc.gpsimd.dma_start(ident[:], ident_d.ap()[:])
        ones = const.tile([CHUNK, 1], BF)
        nc.gpsimd.dma_start(ones[:], ones_d.ap()[:])

        # ---- accumulators: per proposal-half, [hi-part | lo-part] in one bank ----
        ps_a = psA.tile([CHUNK, 2 * R], FP)
        ps_b = psA.tile([CHUNK, 2 * R], FP)
        ps_cnt = psA.tile([1, P], FP)      # counts

        # ---- main loop ----
        feats_h = feats_l = None
        for i in range(N_CHUNKS):
            g, j = divmod(i, GROUP)
            if j == 0:
                feats_g = fpool.tile([CHUNK, GROUP * 2 * R], BF, tag="fg")
                nc.gpsimd.dma_start(
                    feats_g[:],
                    feats_d.ap()[:, g * GROUP * 2 * R : (g + 1) * GROUP * 2 * R],
                )
            # t' = x*a - b (scaled box test) on PE: one ldweights + two matmuls
            lhsT = xyzT[:, i * CHUNK : (i + 1) * CHUNK]
            ps_t = psT.tile([CHUNK, 3 * P], FP, tag="tp")
            nc.tensor.matmul(ps_t[:, 0:512], lhsT, rb[:, 0:512], start=True, stop=True)
            nc.tensor.matmul(ps_t[:, 512:768], lhsT, rb[:, 512:768], start=True, stop=True)
            # |t'| on ACT (PSUM->SBUF), then (<=1) on DVE, AND-combine on GP+DVE
            u3 = cmp_p.tile([CHUNK, 3 * P], FP, tag="u3")
            nc.scalar.activation(u3[:], ps_t[:], Act.Abs, bias=0.0, scale=1.0)
            b3 = cmp_p.tile([CHUNK, 3 * P], BF, tag="b3")
            nc.vector.tensor_scalar(b3[:], u3[:], 1.0, None, Op.is_le)
            m1 = cmp_p.tile([CHUNK, P], BF, tag="m1")
            nc.gpsimd.tensor_tensor(m1[:], b3[:, 0:P], b3[:, P : 2 * P], Op.mult)
            M = mpool.tile([CHUNK, P], BF, tag="M")
            nc.vector.tensor_tensor(M[:], m1[:], b3[:, 2 * P : 3 * P], Op.mult)

            rhs2 = feats_g[:, j * 2 * R : (j + 1) * 2 * R]
            st, sp = i == 0, i == N_CHUNKS - 1
            nc.tensor.matmul(ps_a[:], M[:, 0:CHUNK], rhs2, start=st, stop=sp)
            nc.tensor.matmul(ps_b[:], M[:, CHUNK:P], rhs2, start=st, stop=sp)
            nc.tensor.matmul(ps_cnt[:], ones[:], M[:], start=st, stop=sp)

        # ---- allreduce partials ----
        t0 = spool.tile([CHUNK, R], FP)
        t1 = spool.tile([CHUNK, R], FP)
        s_sb0 = spool.tile([CHUNK, R], FP)
        s_sb1 = spool.tile([CHUNK, R], FP)
        c_sb = spool.tile([1, P], FP)
        nc.vector.tensor_copy(t0[:], ps_a[:, 0:R])
        nc.vector.tensor_copy(t1[:], ps_b[:, 0:R])
        nc.vector.scalar_tensor_tensor(s_sb0[:], ps_a[:, R : 2 * R], 0.0, t0[:], Op.add, Op.add)
        nc.vector.scalar_tensor_tensor(s_sb1[:], ps_b[:, R : 2 * R], 0.0, t1[:], Op.add, Op.add)
        nc.vector.tensor_copy(c_sb[:], ps_cnt[:])

        b_in = dram.tile([P + 1, R], FP)
        b_out = dram.tile([P + 1, R], FP)
        nc.gpsimd.dma_start(b_in[0:CHUNK, :], s_sb0[:])
        nc.gpsimd.dma_start(b_in[CHUNK:P, :], s_sb1[:])
        nc.gpsimd.dma_start(b_in[P : P + 1, :], c_sb[:])
        nc.gpsimd.collective_compute(
            "AllReduce", Op.add,
            replica_groups=[list(range(N_CORES))],
            ins=[b_in.opt()], outs=[b_out.opt()],
        )
        rs0 = spool.tile([CHUNK, R], FP)
        rs1 = spool.tile([CHUNK, R], FP)
        rc = spool.tile([1, P], FP)
        nc.gpsimd.dma_start(rs0[:], b_out[0:CHUNK, :])
        nc.gpsimd.dma_start(rs1[:], b_out[CHUNK:P, :])
        nc.gpsimd.dma_start(rc[:], b_out[P : P + 1, :])

        # ---- head (replicated) ----
        # clamp counts, transpose to columns, reciprocal
        cl = spool.tile([1, P], FP)
        nc.vector.tensor_scalar(cl[:], rc[:], 1.0, None, Op.max)
        ps_small = psS.tile([CHUNK, 4 * CO + 2], FP)
        nc.tensor.transpose(ps_small[:, 4 * CO : 4 * CO + 1], cl[:, 0:CHUNK], ident[0:1, 0:1])
        nc.tensor.transpose(ps_small[:, 4 * CO + 1 : 4 * CO + 2], cl[:, CHUNK:P], ident[0:1, 0:1])
        cc0 = spool.tile([CHUNK, 1], FP)
        cc1 = spool.tile([CHUNK, 1], FP)
        nc.vector.tensor_copy(cc0[:], ps_small[:, 4 * CO : 4 * CO + 1])
        nc.vector.tensor_copy(cc1[:], ps_small[:, 4 * CO + 1 : 4 * CO + 2])
        rcp0 = spool.tile([CHUNK, 1], FP)
        rcp1 = spool.tile([CHUNK, 1], FP)
        nc.vector.reciprocal(rcp0[:], cc0[:])
        nc.vector.reciprocal(rcp1[:], cc1[:])

        roi0 = spool.tile([CHUNK, R], FP)
        roi1 = spool.tile([CHUNK, R], FP)
        nc.vector.tensor_scalar(roi0[:], rs0[:], rcp0[:], None, Op.mult)
        nc.vector.tensor_scalar(roi1[:], rs1[:], rcp1[:], None, Op.mult)

        # transpose roi -> roiT [r, p] halves
        ps_rT01 = psA.tile([CHUNK, 2 * P], FP, tag="ps_a")
        ps_rT0 = ps_rT01[:, 0:P]
        ps_rT1 = ps_rT01[:, P : 2 * P]
        nc.tensor.transpose(ps_rT01[:, 0:CHUNK], roi0[:, 0:CHUNK], ident[:])
        nc.tensor.transpose(ps_rT01[:, CHUNK:P], roi1[:, 0:CHUNK], ident[:])
        nc.tensor.transpose(ps_rT01[:, P : P + CHUNK], roi0[:, CHUNK:R], ident[:])
        nc.tensor.transpose(ps_rT01[:, P + CHUNK : 2 * P], roi1[:, CHUNK:R], ident[:])
        rT0 = spool.tile([CHUNK, P], FPR)
        rT1 = spool.tile([CHUNK, P], FPR)
        nc.vector.tensor_copy(rT0[:], ps_rT0)
        nc.vector.tensor_copy(rT1[:], ps_rT1)

        # logitsT [42, 256] = W_cat.T @ roiT + b
        ps_lg = psS.tile([64, P], FP)
        nc.tensor.matmul(ps_lg[:], wc0[:], rT0[:], start=True, stop=False)
        nc.tensor.matmul(ps_lg[:], wc1[:], rT1[:], start=False, stop=True)
        lg = spool.tile([64, P], FP)
        nc.scalar.activation(lg[:], ps_lg[:], Act.Identity, bias=bcat[:], scale=1.0)

        # obj softmax over proposals (free dim) on rows CO..2*CO
        lgo = lg[32 : 32 + CO, :]
        mo = spool.tile([CO, 1], FP)
        nc.vector.tensor_reduce(mo[:], lgo, AxisListType.X, Op.max)
        nmo = spool.tile([CO, 1], FP)
        nc.vector.tensor_scalar(nmo[:], mo[:], -1.0, None, Op.mult)
        eo = spool.tile([CO, P], FP)
        nc.scalar.activation(eo[:], lgo, Act.Exp, bias=nmo[:], scale=1.0)
        so = spool.tile([CO, 1], FP)
        nc.vector.tensor_reduce(so[:], eo[:], AxisListType.X, Op.add)
        ro = spool.tile([CO, 1], FP)
        nc.vector.reciprocal(ro[:], so[:])
        objp = spool.tile([CO, P], FP)
        nc.vector.tensor_scalar(objp[:], eo[:], ro[:], None, Op.mult)

        # cls: transpose logits rows 0..CO -> [p, c] halves, softmax over c (free)
        ps_cT0 = ps_small[:, 0:CO]
        ps_cT1 = ps_small[:, CO : 2 * CO]
        nc.tensor.transpose(ps_cT0, lg[0:CO, 0:CHUNK], ident[0:CO, 0:CO])
        nc.tensor.transpose(ps_cT1, lg[0:CO, CHUNK:P], ident[0:CO, 0:CO])
        # obj_p transposed -> [p, c] halves
        ps_oT0 = ps_small[:, 2 * CO : 3 * CO]
        ps_oT1 = ps_small[:, 3 * CO : 4 * CO]
        nc.tensor.transpose(ps_oT0, objp[:, 0:CHUNK], ident[0:CO, 0:CO])
        nc.tensor.transpose(ps_oT1, objp[:, CHUNK:P], ident[0:CO, 0:CO])

        for h, (ps_cT, ps_oT) in enumerate(((ps_cT0, ps_oT0), (ps_cT1, ps_oT1))):
            ct = spool.tile([CHUNK, CO], FP, tag=f"ct{h}")
            nc.vector.tensor_copy(ct[:], ps_cT)
            mc = spool.tile([CHUNK, 1], FP, tag=f"mc{h}")
            nc.vector.tensor_reduce(mc[:], ct[:], AxisListType.X, Op.max)
            nmc = spool.tile([CHUNK, 1], FP, tag=f"nmc{h}")
            nc.vector.tensor_scalar(nmc[:], mc[:], -1.0, None, Op.mult)
            ec = spool.tile([CHUNK, CO], FP, tag=f"ec{h}")
            nc.scalar.activation(ec[:], ct[:], Act.Exp, bias=nmc[:], scale=1.0)
            sc = spool.tile([CHUNK, 1], FP, tag=f"sc{h}")
            nc.vector.tensor_reduce(sc[:], ec[:], AxisListType.X, Op.add)
            rc2 = spool.tile([CHUNK, 1], FP, tag=f"rc2{h}")
            nc.vector.reciprocal(rc2[:], sc[:])
            clsp = spool.tile([CHUNK, CO], FP, tag=f"clsp{h}")
            nc.vector.tensor_scalar(clsp[:], ec[:], rc2[:], None, Op.mult)
            outh = spool.tile([CHUNK, CO], FP, tag=f"outh{h}")
            nc.vector.tensor_tensor(outh[:], clsp[:], ps_oT, Op.mult)
            nc.gpsimd.dma_start(out_d.ap()[h * CHUNK : (h + 1) * CHUNK, :], outh[:])

    nc.compile()
    return nc


def _round12(a):
    m, e = np.frexp(a.astype(np.float32))
    return (np.round(m * 4096.0) / 4096.0 * np.exp2(e.astype(np.float32))).astype(
        np.float32
    )


def kernel(
    proposals, input_xyz, seg_feats, W_cls, b_cls, W_obj, b_obj, _trace=False
):
    if "nc" not in _cache:
        _cache["nc"] = _build()
    nc = _cache["nc"]

    proposals = np.asarray(proposals, dtype=np.float32)
    input_xyz = np.asarray(input_xyz, dtype=np.float32)
    seg_feats = np.asarray(seg_feats, dtype=np.float32)

    ctr = proposals[:, :3]
    half = proposals[:, 3:] * np.float32(0.5)
    lo = ctr - half
    hi = ctr + half

    def r12(a):
        a = np.asarray(a, np.float32)
        m, e = np.frexp(a)
        return (np.round(m * 4096.0) / 4096.0 * np.exp2(e.astype(np.float32))).astype(np.float32)

    lo64, hi64 = lo.astype(np.float64), hi.astype(np.float64)
    a64 = 2.0 / (hi64 - lo64)
    b64 = (hi64 + lo64) / (hi64 - lo64)
    ah = r12(a64.astype(np.float32))
    al = r12((a64 - ah.astype(np.float64)).astype(np.float32))
    bh = r12(b64.astype(np.float32))
    bl = r12((b64 - bh.astype(np.float64)).astype(np.float32))

    rb = np.zeros((14, 3 * P), np.float32)
    for d in range(3):
        c0, c1 = d * P, (d + 1) * P
        rb[4 * d + 0, c0:c1] = ah[:, d]
        rb[4 * d + 1, c0:c1] = ah[:, d]
        rb[4 * d + 2, c0:c1] = al[:, d]
        rb[4 * d + 3, c0:c1] = al[:, d]
        rb[12, c0:c1] = -bh[:, d]
        rb[13, c0:c1] = -bl[:, d]

    n_tot = N_CORES * N_PAD_CORE
    xyz_pad = np.full((n_tot, 3), 9.0, np.float32)
    xyz_pad[:N] = input_xyz
    feats_pad = np.zeros((n_tot, R), np.float32)
    feats_pad[:N] = seg_feats

    import ml_dtypes
    wcat = np.zeros((R, 64), np.float32)
    wcat[:, 0:CO] = W_cls
    wcat[:, 32 : 32 + CO] = W_obj
    bcat = np.zeros((64, 1), np.float32)
    bcat[0:CO, 0] = b_cls
    bcat[32 : 32 + CO, 0] = b_obj
    ident = np.eye(CHUNK, dtype=np.float32)
    ones = np.ones((CHUNK, 1), ml_dtypes.bfloat16)
    BF_NP = ml_dtypes.bfloat16

    in_maps = []
    for c in range(N_CORES):
        sl = slice(c * N_PAD_CORE, (c + 1) * N_PAD_CORE)
        xs = xyz_pad[sl]
        xh = r12(xs)
        xl = r12(xs - xh)
        xyzT = np.ones((14, N_PAD_CORE), np.float32)
        for d in range(3):
            xyzT[4 * d + 0] = xh[:, d]
            xyzT[4 * d + 1] = xl[:, d]
            xyzT[4 * d + 2] = xh[:, d]
            xyzT[4 * d + 3] = xl[:, d]
        fc = (
            feats_pad[sl].reshape(N_CHUNKS, CHUNK, R).transpose(1, 0, 2)
            .reshape(CHUNK, N_CHUNKS, R)
        )
        fh = fc.astype(BF_NP)
        fl = (fc - fh.astype(np.float32)).astype(BF_NP)
        fm = np.concatenate([fh, fl], axis=2).reshape(CHUNK, N_CHUNKS * 2 * R)
        in_maps.append(
            {
                "feats": fm, "xyzT": xyzT, "rb": rb,
                "wcat": wcat, "bcat": bcat, "ident": ident, "ones": ones,
            }
        )

    res = run_bass_kernel_spmd(
        nc, in_maps, core_ids=list(range(N_CORES)), trace=_trace
    )
    out = res.results[0]["out"]
    if _trace:
        _cache["last_exec_ns"] = res.exec_time_ns
        _cache["last_results"] = res
    return out


# revision 10
# speedup vs baseline: 1.2075x; 1.2075x over previous
"""Trainium2 Bass kernel for nn_Pointnet2DetHead (segment_reduce).

Pipeline per core (N sharded 8 ways, 12544 points/core in 98 chunks of 128):
  - box-membership mask[n,p] via exact fp32 compares:
      x-dim on DVE (tensor_scalar is_le/is_ge -> {0,2} int16)
      y/z dims on ACT (Sign(coord - lo), Sign(hi - coord) -> {-1,0,1} int16)
      combined with int16 scalar_tensor_tensor adds + threshold >= 7
  - sums[p,r] and counts[p] via float32r matmuls accumulated in PSUM
  - AllReduce partials across the 8 cores
  - replicated head: mean -> linear heads -> softmax(cls, axis=c) *
    softmax(obj, axis=p)
"""

import sys

if "/opt/trn_rl_repo" not in sys.path:
    sys.path.insert(0, "/opt/trn_rl_repo")

import numpy as np
from contextlib import ExitStack

import concourse.bass as bass
import concourse.tile as tile
import concourse.mybir as mybir
from concourse import bacc
from concourse.bass_utils import run_bass_kernel_spmd
from concourse.mybir import AluOpType as Op, ActivationFunctionType as Act, AxisListType

N_CORES = 8
N, P, R, CO = 100000, 256, 256, 21  # CO = C+1 output classes
CHUNK = 128
N_PAD_CORE = 12544            # 98 chunks of 128
N_CHUNKS = N_PAD_CORE // CHUNK
GROUP = 7                     # chunks per feats DMA group
N_GROUPS = N_CHUNKS // GROUP
FP = mybir.dt.float32
FPR = mybir.dt.float32r
I16 = mybir.dt.int16
BF = mybir.dt.bfloat16

_cache = {}


def _build():
    nc = bacc.Bacc("TRN2", num_devices=N_CORES, target_bir_lowering=False, debug=False)

    feats_d = nc.dram_tensor("feats", [CHUNK, N_CHUNKS * 2 * R], BF, kind="ExternalInput")
    xyzT_d = nc.dram_tensor("xyzT", [14, N_PAD_CORE], FPR, kind="ExternalInput")
    rb_d = nc.dram_tensor("rb", [14, 3 * P], FPR, kind="ExternalInput")
    wcat_d = nc.dram_tensor("wcat", [2 * CHUNK, 64], FPR, kind="ExternalInput")
    bcat_d = nc.dram_tensor("bcat", [64, 1], FP, kind="ExternalInput")
    ident_d = nc.dram_tensor("ident", [CHUNK, CHUNK], FP, kind="ExternalInput")
    ones_d = nc.dram_tensor("ones", [CHUNK, 1], BF, kind="ExternalInput")
    out_d = nc.dram_tensor("out", [P, CO], FP, kind="ExternalOutput")

    with ExitStack() as ctx:
        tc = ctx.enter_context(tile.TileContext(nc))
        const = ctx.enter_context(tc.tile_pool(name="const", bufs=1))
        fpool = ctx.enter_context(tc.tile_pool(name="fpool", bufs=4))
        cmp_p = ctx.enter_context(tc.tile_pool(name="cmp", bufs=6))
        mpool = ctx.enter_context(tc.tile_pool(name="mp", bufs=6))
        spool = ctx.enter_context(tc.tile_pool(name="sp", bufs=2))
        psA = ctx.enter_context(tc.tile_pool(name="psA", bufs=1, space="PSUM"))
        psT = ctx.enter_context(tc.tile_pool(name="psT", bufs=2, space="PSUM"))
        psS = ctx.enter_context(tc.tile_pool(name="psS", bufs=1, space="PSUM"))
        dram = ctx.enter_context(tc.tile_pool(name="dram", bufs=1, space="DRAM"))

        # ---- constants ----
        xyzT = const.tile([14, N_PAD_CORE], FPR)  # [xh,xl,xh, yh,yl,yh, zh,zl,zh, 1,1]
        nc.gpsimd.dma_start(xyzT[:], xyzT_d.ap()[:])
        rb = const.tile([14, 3 * P], FPR)         # scaled-bound matmul rhs
        nc.gpsimd.dma_start(rb[:], rb_d.ap()[:])
        wc0 = const.tile([CHUNK, 64], FPR)
        wc1 = const.tile([CHUNK, 64], FPR)
        nc.gpsimd.dma_start(wc0[:], wcat_d.ap()[0:CHUNK, :])
        nc.gpsimd.dma_start(wc1[:], wcat_d.ap()[CHUNK : 2 * CHUNK, :])
        bcat = const.tile([64, 1], FP)
        nc.gpsimd.dma_start(bcat[:], bcat_d.ap()[:])
        ident = const.tile([CHUNK, CHUNK], FP)
        nc.gpsimd.dma_start(ident[:], ident_d.ap()[:])
        ones = const.tile([CHUNK, 1], BF)
        nc.gpsimd.dma_start(ones[:], ones_d.ap()[:])

        # ---- accumulators: per proposal-half, [hi-part | lo-part] in one bank ----
        ps_a = psA.tile([CHUNK, 2 * R], FP)
        ps_b = psA.tile([CHUNK, 2 * R], FP)
        ps_cnt = psS.tile([1, P], FP, tag="cntlg")  # counts (bank later reused by lg)

        # ---- main loop ----
        feats_h = feats_l = None
        for i in range(N_CHUNKS):
            g, j = divmod(i, GROUP)
            if j == 0:
                feats_g = fpool.tile([CHUNK, GROUP * 2 * R], BF, tag="fg")
                nc.gpsimd.dma_start(
                    feats_g[:],
                    feats_d.ap()[:, g * GROUP * 2 * R : (g + 1) * GROUP * 2 * R],
                )
            # t' = x*a - b (scaled box test) on PE: one ldweights + two matmuls
            lhsT = xyzT[:, i * CHUNK : (i + 1) * CHUNK]
            ps_t = psT.tile([CHUNK, 3 * P], FP, tag="tp")
            nc.tensor.matmul(ps_t[:, 0:512], lhsT, rb[:, 0:512], start=True, stop=True)
            nc.tensor.matmul(ps_t[:, 512:768], lhsT, rb[:, 512:768], start=True, stop=True)
            # |t'| on ACT (PSUM->SBUF), then (<=1) on DVE, AND-combine on GP+DVE
            u3 = cmp_p.tile([CHUNK, 3 * P], FP, tag="u3")
            nc.scalar.activation(u3[:], ps_t[:], Act.Abs, bias=0.0, scale=1.0)
            b3 = cmp_p.tile([CHUNK, 3 * P], BF, tag="b3")
            nc.vector.tensor_scalar(b3[:], u3[:], 1.0, None, Op.is_le)
            m1 = cmp_p.tile([CHUNK, P], BF, tag="m1")
            nc.gpsimd.tensor_tensor(m1[:], b3[:, 0:P], b3[:, P : 2 * P], Op.mult)
            M = mpool.tile([CHUNK, P], BF, tag="M")
            nc.vector.tensor_tensor(M[:], m1[:], b3[:, 2 * P : 3 * P], Op.mult)

            rhs2 = feats_g[:, j * 2 * R : (j + 1) * 2 * R]
            st, sp = i == 0, i == N_CHUNKS - 1
            nc.tensor.matmul(ps_a[:], M[:, 0:CHUNK], rhs2, start=st, stop=sp)
            nc.tensor.matmul(ps_b[:], M[:, CHUNK:P], rhs2, start=st, stop=sp)
            nc.tensor.matmul(ps_cnt[:], ones[:], M[:], start=st, stop=sp)

        # ---- allreduce partials ----
        t0 = spool.tile([CHUNK, R], FP)
        t1 = spool.tile([CHUNK, R], FP)
        s_sb0 = spool.tile([CHUNK, R], FP)
        s_sb1 = spool.tile([CHUNK, R], FP)
        c_sb = spool.tile([1, P], FP)
        nc.vector.tensor_copy(t0[:], ps_a[:, 0:R])
        nc.vector.tensor_copy(t1[:], ps_b[:, 0:R])
        nc.vector.scalar_tensor_tensor(s_sb0[:], ps_a[:, R : 2 * R], 0.0, t0[:], Op.add, Op.add)
        nc.vector.scalar_tensor_tensor(s_sb1[:], ps_b[:, R : 2 * R], 0.0, t1[:], Op.add, Op.add)
        nc.vector.tensor_copy(c_sb[:], ps_cnt[:])

        b_in = dram.tile([P + 1, R], FP)
        b_out = dram.tile([P + 1, R], FP)
        nc.gpsimd.dma_start(b_in[0:CHUNK, :], s_sb0[:])
        nc.gpsimd.dma_start(b_in[CHUNK:P, :], s_sb1[:])
        nc.gpsimd.dma_start(b_in[P : P + 1, :], c_sb[:])
        nc.gpsimd.collective_compute(
            "AllReduce", Op.add,
            replica_groups=[list(range(N_CORES))],
            ins=[b_in.opt()], outs=[b_out.opt()],
        )
        rs0 = spool.tile([CHUNK, R], FP)
        rs1 = spool.tile([CHUNK, R], FP)
        rc = spool.tile([1, P], FP)
        nc.gpsimd.dma_start(rs0[:], b_out[0:CHUNK, :])
        nc.gpsimd.dma_start(rs1[:], b_out[CHUNK:P, :])
        nc.gpsimd.dma_start(rc[:], b_out[P : P + 1, :])

        # ---- head (replicated) ----
        # clamp counts, transpose to columns, reciprocal
        cl = spool.tile([1, P], FP)
        nc.vector.tensor_scalar(cl[:], rc[:], 1.0, None, Op.max)
        ps_small = psS.tile([CHUNK, 4 * CO + 2], FP)
        nc.tensor.transpose(ps_small[:, 4 * CO : 4 * CO + 1], cl[:, 0:CHUNK], ident[0:1, 0:1])
        nc.tensor.transpose(ps_small[:, 4 * CO + 1 : 4 * CO + 2], cl[:, CHUNK:P], ident[0:1, 0:1])
        cc0 = spool.tile([CHUNK, 1], FP)
        cc1 = spool.tile([CHUNK, 1], FP)
        nc.vector.tensor_copy(cc0[:], ps_small[:, 4 * CO : 4 * CO + 1])
        nc.vector.tensor_copy(cc1[:], ps_small[:, 4 * CO + 1 : 4 * CO + 2])
        rcp0 = spool.tile([CHUNK, 1], FP)
        rcp1 = spool.tile([CHUNK, 1], FP)
        nc.vector.reciprocal(rcp0[:], cc0[:])
        nc.vector.reciprocal(rcp1[:], cc1[:])

        roi0 = spool.tile([CHUNK, R], FP)
        roi1 = spool.tile([CHUNK, R], FP)
        nc.vector.tensor_scalar(roi0[:], rs0[:], rcp0[:], None, Op.mult)
        nc.vector.tensor_scalar(roi1[:], rs1[:], rcp1[:], None, Op.mult)

        # transpose roi -> roiT [r, p] halves
        ps_rT01 = psA.tile([CHUNK, 2 * P], FP, tag="ps_a")
        ps_rT0 = ps_rT01[:, 0:P]
        ps_rT1 = ps_rT01[:, P : 2 * P]
        nc.tensor.transpose(ps_rT01[:, 0:CHUNK], roi0[:, 0:CHUNK], ident[:])
        nc.tensor.transpose(ps_rT01[:, CHUNK:P], roi1[:, 0:CHUNK], ident[:])
        nc.tensor.transpose(ps_rT01[:, P : P + CHUNK], roi0[:, CHUNK:R], ident[:])
        nc.tensor.transpose(ps_rT01[:, P + CHUNK : 2 * P], roi1[:, CHUNK:R], ident[:])
        rT0 = spool.tile([CHUNK, P], FPR)
        rT1 = spool.tile([CHUNK, P], FPR)
        nc.vector.tensor_copy(rT0[:], ps_rT0)
        nc.vector.tensor_copy(rT1[:], ps_rT1)

        # logitsT [42, 256] = W_cat.T @ roiT + b
        ps_lg = psS.tile([64, P], FP, tag="cntlg")
        nc.tensor.matmul(ps_lg[:], wc0[:], rT0[:], start=True, stop=False)
        nc.tensor.matmul(ps_lg[:], wc1[:], rT1[:], start=False, stop=True)
        lg = spool.tile([64, P], FP)
        nc.scalar.activation(lg[:], ps_lg[:], Act.Identity, bias=bcat[:], scale=1.0)

        # obj softmax over proposals (free dim) on rows CO..2*CO
        lgo = lg[32 : 32 + CO, :]
        mo = spool.tile([CO, 1], FP)
        nc.vector.tensor_reduce(mo[:], lgo, AxisListType.X, Op.max)
        nmo = spool.tile([CO, 1], FP)
        nc.vector.tensor_scalar(nmo[:], mo[:], -1.0, None, Op.mult)
        eo = spool.tile([CO, P], FP)
        nc.scalar.activation(eo[:], lgo, Act.Exp, bias=nmo[:], scale=1.0)
        so = spool.tile([CO, 1], FP)
        nc.vector.tensor_reduce(so[:], eo[:], AxisListType.X, Op.add)
        ro = spool.tile([CO, 1], FP)
        nc.vector.reciprocal(ro[:], so[:])
        objp = spool.tile([CO, P], FP)
        nc.vector.tensor_scalar(objp[:], eo[:], ro[:], None, Op.mult)

        # cls: transpose logits rows 0..CO -> [p, c] halves, softmax over c (free)
        ps_cT0 = ps_small[:, 0:CO]
        ps_cT1 = ps_small[:, CO : 2 * CO]
        nc.tensor.transpose(ps_cT0, lg[0:CO, 0:CHUNK], ident[0:CO, 0:CO])
        nc.tensor.transpose(ps_cT1, lg[0:CO, CHUNK:P], ident[0:CO, 0:CO])
        # obj_p transposed -> [p, c] halves
        ps_oT0 = ps_small[:, 2 * CO : 3 * CO]
        ps_oT1 = ps_small[:, 3 * CO : 4 * CO]
        nc.tensor.transpose(ps_oT0, objp[:, 0:CHUNK], ident[0:CO, 0:CO])
        nc.tensor.transpose(ps_oT1, objp[:, CHUNK:P], ident[0:CO, 0:CO])

        for h, (ps_cT, ps_oT) in enumerate(((ps_cT0, ps_oT0), (ps_cT1, ps_oT1))):
            ct = spool.tile([CHUNK, CO], FP, tag=f"ct{h}")
            nc.vector.tensor_copy(ct[:], ps_cT)
            mc = spool.tile([CHUNK, 1], FP, tag=f"mc{h}")
            nc.vector.tensor_reduce(mc[:], ct[:], AxisListType.X, Op.max)
            nmc = spool.tile([CHUNK, 1], FP, tag=f"nmc{h}")
            nc.vector.tensor_scalar(nmc[:], mc[:], -1.0, None, Op.mult)
            ec = spool.tile([CHUNK, CO], FP, tag=f"ec{h}")
            nc.scalar.activation(ec[:], ct[:], Act.Exp, bias=nmc[:], scale=1.0)
            sc = spool.tile([CHUNK, 1], FP, tag=f"sc{h}")
            nc.vector.tensor_reduce(sc[:], ec[:], AxisListType.X, Op.add)
            rc2 = spool.tile([CHUNK, 1], FP, tag=f"rc2{h}")
            nc.vector.reciprocal(rc2[:], sc[:])
            clsp = spool.tile([CHUNK, CO], FP, tag=f"clsp{h}")
            nc.vector.tensor_scalar(clsp[:], ec[:], rc2[:], None, Op.mult)
            outh = spool.tile([CHUNK, CO], FP, tag=f"outh{h}")
            nc.vector.tensor_tensor(outh[:], clsp[:], ps_oT, Op.mult)
            nc.gpsimd.dma_start(out_d.ap()[h * CHUNK : (h + 1) * CHUNK, :], outh[:])

    nc.compile()
    return nc


def _round12(a):
    m, e = np.frexp(a.astype(np.float32))
    return (np.round(m * 4096.0) / 4096.0 * np.exp2(e.astype(np.float32))).astype(
        np.float32
    )


def kernel(
    proposals, input_xyz, seg_feats, W_cls, b_cls, W_obj, b_obj, _trace=False
):
    if "nc" not in _cache:
        _cache["nc"] = _build()
    nc = _cache["nc"]

    proposals = np.asarray(proposals, dtype=np.float32)
    input_xyz = np.asarray(input_xyz, dtype=np.float32)
    seg_feats = np.asarray(seg_feats, dtype=np.float32)

    ctr = proposals[:, :3]
    half = proposals[:, 3:] * np.float32(0.5)
    lo = ctr - half
    hi = ctr + half

    def r12(a):
        a = np.asarray(a, np.float32)
        m, e = np.frexp(a)
        return (np.round(m * 4096.0) / 4096.0 * np.exp2(e.astype(np.float32))).astype(np.float32)

    lo64, hi64 = lo.astype(np.float64), hi.astype(np.float64)
    a64 = 2.0 / (hi64 - lo64)
    b64 = (hi64 + lo64) / (hi64 - lo64)
    ah = r12(a64.astype(np.float32))
    al = r12((a64 - ah.astype(np.float64)).astype(np.float32))
    bh = r12(b64.astype(np.float32))
    bl = r12((b64 - bh.astype(np.float64)).astype(np.float32))

    rb = np.zeros((14, 3 * P), np.float32)
    for d in range(3):
        c0, c1 = d * P, (d + 1) * P
        rb[4 * d + 0, c0:c1] = ah[:, d]
        rb[4 * d + 1, c0:c1] = ah[:, d]
        rb[4 * d + 2, c0:c1] = al[:, d]
        rb[4 * d + 3, c0:c1] = al[:, d]
        rb[12, c0:c1] = -bh[:, d]
        rb[13, c0:c1] = -bl[:, d]

    n_tot = N_CORES * N_PAD_CORE
    xyz_pad = np.full((n_tot, 3), 9.0, np.float32)
    xyz_pad[:N] = input_xyz
    feats_pad = np.zeros((n_tot, R), np.float32)
    feats_pad[:N] = seg_feats

    import ml_dtypes
    wcat = np.zeros((R, 64), np.float32)
    wcat[:, 0:CO] = W_cls
    wcat[:, 32 : 32 + CO] = W_obj
    bcat = np.zeros((64, 1), np.float32)
    bcat[0:CO, 0] = b_cls
    bcat[32 : 32 + CO, 0] = b_obj
    ident = np.eye(CHUNK, dtype=np.float32)
    ones = np.ones((CHUNK, 1), ml_dtypes.bfloat16)
    BF_NP = ml_dtypes.bfloat16

    in_maps = []
    for c in range(N_CORES):
        sl = slice(c * N_PAD_CORE, (c + 1) * N_PAD_CORE)
        xs = xyz_pad[sl]
        xh = r12(xs)
        xl = r12(xs - xh)
        xyzT = np.ones((14, N_PAD_CORE), np.float32)
        for d in range(3):
            xyzT[4 * d + 0] = xh[:, d]
            xyzT[4 * d + 1] = xl[:, d]
            xyzT[4 * d + 2] = xh[:, d]
            xyzT[4 * d + 3] = xl[:, d]
        fc = (
            feats_pad[sl].reshape(N_CHUNKS, CHUNK, R).transpose(1, 0, 2)
            .reshape(CHUNK, N_CHUNKS, R)
        )
        fh = fc.astype(BF_NP)
        fl = (fc - fh.astype(np.float32)).astype(BF_NP)
        fm = np.concatenate([fh, fl], axis=2).reshape(CHUNK, N_CHUNKS * 2 * R)
        in_maps.append(
            {
                "feats": fm, "xyzT": xyzT, "rb": rb,
                "wcat": wcat, "bcat": bcat, "ident": ident, "ones": ones,
            }
        )

    res = run_bass_kernel_spmd(
        nc, in_maps, core_ids=list(range(N_CORES)), trace=_trace
    )
    out = res.results[0]["out"]
    if _trace:
        _cache["last_exec_ns"] = res.exec_time_ns
        _cache["last_results"] = res
    return out
